# revision 4
# baseline (speedup 1.0000x reference)
"""Trainium2 Bass kernel for nn_AudioRNN (LSTM(13->32, T=25) + FC(32->4), B=65536).

Strategy v2 (pure data parallel over batch, 8 cores x 8192 rows):

  * Same chunk-quadrant elementwise layout as v1: superchunk = 4 chunks of
    512 batch rows; chunk c occupies SBUF/PSUM partition quadrant c (32
    partitions = 32 hidden dims); gate pre-activations for one (t, sc) in one
    PSUM tile [128, 4*512] with free-dim bank G = gate G (f, i, o, g).
  * v2 matmuls are full-PE-array with BLOCK-DIAGONAL stationary weights so a
    single instruction covers all 4 chunks:
      - input proj, gate g: lhsT [56, 128], block j = rows 14j..14j+14 x cols
        32j..32j+32 holding [W_ih_g; b_g]^T; rhs = xt tile [56, 512] with
        chunk j's (x_t, ones) on rows 14j..14j+14.  4 instructions/(t, sc).
      - recurrence, gate g: lhsT [128, 128], block j = W_hh_g^T on the
        diagonal; rhs = h tile [128, 512] as produced by the elementwise
        stage.  4 instructions/(t, sc), accumulated into the same PSUM bank.
    => 8 matmuls x 512 free rows per (t, sc) instead of v1's 32.
  * FC: one block-diag [128, 128] matmul per superchunk (W_fc zero-padded).
"""

import numpy as np
import ml_dtypes

I_DIM = 13
H_DIM = 32
C_DIM = 4
T_STEPS = 25
B_FULL = 65536

N_TG = (T_STEPS + 3) // 4    # 7 t-groups of up to 4 timesteps
KX = I_DIM + 1               # 14: 13 input dims + ones row for bias
KB = 4 * KX                  # 56: stacked input rows for 4 chunks

# free-dim bank order of the gates: f, i, o, g  (sigmoid on banks 0..2, tanh on 3)
# -> PyTorch row-chunk order in W_ih/W_hh is i(0), f(1), g(2), o(3)
GATE_PERM = [1, 0, 3, 2]     # bank G -> pytorch gate chunk index

N_CORES = 8
CH_B = 512                   # batch rows per chunk (= one PSUM bank of fp32)
N_SC = 4                     # superchunks per core

_BF16 = ml_dtypes.bfloat16

_NC_CACHE = {}


def _build_bass(n_sc=N_SC, ch_b=CH_B, split_waits=True):
    import concourse.bass as bass
    import concourse.mybir as mybir
    from concourse.tile import TileContext

    dt = mybir.dt
    AF = mybir.ActivationFunctionType

    sc_b = 4 * ch_b
    b_core = n_sc * sc_b

    nc = bass.Bass("TRN2")

    # xt[s, tg, :, u, :]: rows 14j+r = chunk j input row r at t = 4*tg+u
    xt_d = nc.dram_tensor("xt", [n_sc, N_TG, KB, 4, ch_b], dt.bfloat16,
                          kind="ExternalInput")
    wx_d = nc.dram_tensor("wx", [KB, 512], dt.bfloat16, kind="ExternalInput")
    wh_d = nc.dram_tensor("wh", [128, 512], dt.bfloat16, kind="ExternalInput")
    wfc_d = nc.dram_tensor("wfc", [128, 128], dt.bfloat16,
                           kind="ExternalInput")
    bfc_d = nc.dram_tensor("bfc", [128, 1], dt.float32, kind="ExternalInput")
    out_d = nc.dram_tensor("out", [b_core, C_DIM], dt.float32,
                           kind="ExternalOutput")

    c_dt = dt.bfloat16  # dtype of the cell state c

    with TileContext(nc) as tc:
        with (
            tc.tile_pool(name="singles", bufs=1) as singles,
            tc.tile_pool(name="xt", bufs=n_sc * N_TG) as xt_pool,
            tc.tile_pool(name="sig", bufs=6) as sig_pool,
            tc.tile_pool(name="cell", bufs=8) as cell_pool,
            tc.tile_pool(name="hid", bufs=8) as hid_pool,
            tc.tile_pool(name="tmp", bufs=8) as tmp_pool,
            tc.tile_pool(name="outp", bufs=4) as out_pool,
            tc.tile_pool(name="psum", bufs=2, space="PSUM") as psum_pool,
            tc.tile_pool(name="psumg", bufs=2, space="PSUM") as psumg_pool,
        ):
            wx = singles.tile([KB, 512], dt.bfloat16)
            wh = singles.tile([128, 512], dt.bfloat16)
            wfc = singles.tile([128, 128], dt.bfloat16)
            bfc = singles.tile([128, 1], dt.float32)
            nc.sync.dma_start(out=wx, in_=wx_d[:, :])
            nc.sync.dma_start(out=wh, in_=wh_d[:, :])
            nc.sync.dma_start(out=wfc, in_=wfc_d[:, :])
            nc.sync.dma_start(out=bfc, in_=bfc_d[:, :])

            h_prev = [None] * n_sc
            c_prev = [None] * n_sc
            xt_cur = [None] * n_sc

            rounds = [list(range(n_sc))]
            for rnd in rounds:
              for t in range(T_STEPS):
                tg, u = divmod(t, 4)
                for s in rnd:
                    # -- stage the pre-packed x for this t-group
                    if u == 0:
                        xt = xt_pool.tile([KB, 4 * ch_b], dt.bfloat16,
                                          tag="xt")
                        eng = (nc.sync, nc.gpsimd)[s % 2]
                        eng.dma_start(
                            out=xt, in_=xt_d[s, tg].rearrange("p u b -> p (u b)"))
                        xt_cur[s] = xt
                    xr = xt_cur[s]

                    # -- gate pre-activations: a 3-bank (f,i,o) PSUM tile
                    # and a 1-bank (g) PSUM tile, each freed by its single
                    # ACT reader so the slots recycle early.
                    P = psum_pool.tile([128, 3 * ch_b], dt.float32,
                                       tag="gates")
                    Pg = psumg_pool.tile([128, ch_b], dt.float32, tag="g")
                    for g in range(4):
                        dst = Pg if g == 3 else P[:, ch_b * g:ch_b * (g + 1)]
                        nc.tensor.matmul(
                            out=dst,
                            lhsT=wx[:, 128 * g:128 * g + 128],
                            rhs=xr[:, ch_b * u:ch_b * (u + 1)],
                            start=True,
                            stop=(t == 0),
                            skip_group_check=True,
                        )
                    if t > 0:
                        for g in range(4):
                            dst = Pg if g == 3 else P[:, ch_b * g:ch_b * (g + 1)]
                            nc.tensor.matmul(
                                out=dst,
                                lhsT=wh[:, 128 * g:128 * g + 128],
                                rhs=h_prev[s],
                                start=False,
                                stop=True,
                                skip_group_check=True,
                            )

                    # -- activations: sigmoid(f,i,o) in one op, tanh(g)
                    S = sig_pool.tile([128, 3 * ch_b], dt.bfloat16, tag="S")
                    nc.scalar.activation(out=S, in_=P[:, 0:3 * ch_b],
                                         func=AF.Sigmoid)
                    Gt = tmp_pool.tile([128, ch_b], dt.bfloat16, tag="Gt")
                    nc.scalar.activation(out=Gt, in_=Pg,
                                         func=AF.Tanh)

                    # -- cell update (all lane-aligned, 128 partitions busy)
                    Cn = cell_pool.tile([128, ch_b], c_dt, tag="C")
                    if t == 0:
                        nc.vector.tensor_mul(Cn, S[:, ch_b:2 * ch_b], Gt)
                    else:
                        FCt = tmp_pool.tile([128, ch_b], c_dt, tag="FCt")
                        IGt = tmp_pool.tile([128, ch_b], c_dt, tag="IGt")
                        nc.vector.tensor_mul(FCt, S[:, 0:ch_b], c_prev[s])
                        nc.vector.tensor_mul(IGt, S[:, ch_b:2 * ch_b], Gt)
                        nc.vector.tensor_add(Cn, FCt, IGt)
                    Tc = tmp_pool.tile([128, ch_b], dt.bfloat16, tag="Tc")
                    nc.scalar.activation(out=Tc, in_=Cn, func=AF.Tanh)
                    Hn = hid_pool.tile([128, ch_b], dt.bfloat16, tag="H")
                    nc.vector.tensor_mul(Hn, S[:, 2 * ch_b:3 * ch_b], Tc)
                    c_prev[s] = Cn
                    h_prev[s] = Hn

                    # -- final FC + bias + store, once per superchunk
                    if t == T_STEPS - 1:
                        PF = psumg_pool.tile([128, ch_b], dt.float32,
                                             tag="g")
                        nc.tensor.matmul(
                            out=PF,
                            lhsT=wfc,
                            rhs=Hn,
                            start=True,
                            stop=True,
                            skip_group_check=True,
                        )
                        Ot = out_pool.tile([128, ch_b], dt.float32, tag="O")
                        nc.scalar.add(Ot, PF, bfc)
                        for c in range(4):
                            r0 = s * sc_b + c * ch_b
                            dst = out_d[r0:r0 + ch_b, :].rearrange(
                                "b m -> m b")
                            eng = (nc.gpsimd, nc.sync,
                                   nc.gpsimd, nc.sync)[c]
                            eng.dma_start(
                                out=dst, in_=Ot[32 * c:32 * c + C_DIM, :])

    if split_waits:
        _split_multi_waits(nc, mybir)
    return nc


def _split_multi_waits(nc, mybir):
    """This walrus build allows only ONE sync-wait command per ISA
    instruction.  Tile sometimes emits 2+ (its wait minimization is not
    transitive across processors).  Hoist all-but-one wait onto standalone
    EventSemaphore instructions injected just before, on the same engine —
    semantically identical (the engine stream blocks at the wait either way).
    """
    n_split = 0
    for fn in nc.m.functions:
        for blk in fn.blocks:
            out = []
            for inst in blk.instructions:
                si = getattr(inst, "sync_info", None)
                ow = list(si.on_wait) if si is not None and si.on_wait else []
                if len(ow) > 1 and inst.opcode == "DMACopy" \
                        and str(inst.engine) in ("EngineType.SP",
                                                 "EngineType.Activation"):
                    # Keep the HWDGE queue-slot wait on the DMA descriptor;
                    # hoist data-dependency waits onto the engine stream
                    # (SP blocks before issuing the descriptor - a strictly
                    # stronger ordering, so semantically safe).
                    qw = [w for w in ow if "DMA" in (w.ant_name or "")]
                    rest = [w for w in ow if "DMA" not in (w.ant_name or "")]
                    ow = rest + (qw[-1:] if qw else rest[-1:])
                    ow = rest + qw[-1:] if qw else rest
                if len(ow) > 1:
                    for w in ow[:-1]:
                        n_split += 1
                        ev = mybir.InstEventSemaphore(
                            name=f"splitw-{n_split}-{inst.name}",
                            engine=inst.engine,
                            ins=[],
                            outs=[],
                            sync_info=mybir.SyncInfo(on_wait=[w],
                                                     on_update=[]),
                            bass_priority=inst.bass_priority,
                            bass_scheduled_tick=inst.bass_scheduled_tick,
                            bass_scheduled_proc=inst.bass_scheduled_proc,
                            bass_scheduled_scope=inst.bass_scheduled_scope,
                        )
                        nc.inst_map[ev.name] = ev
                        out.append(ev)
                    si.on_wait = ow[-1:]
                out.append(inst)
            blk.instructions = out
    return n_split


def _get_nc():
    if "nc" not in _NC_CACHE:
        _NC_CACHE["nc"] = _build_bass()
    return _NC_CACHE["nc"]


def _prep_core_inputs(x_core, weight_arrs, n_sc=N_SC, ch_b=CH_B):
    """x_core: [b_core, T, I] fp32 -> the per-core input map."""
    # [sc, chunk j, b, t, i]
    xr = x_core.reshape(n_sc, 4, ch_b, T_STEPS, I_DIM)
    # -> [sc, t, j, i, b]
    xf = np.ascontiguousarray(xr.transpose(0, 3, 1, 4, 2))
    xt = np.zeros((n_sc, N_TG, 4, KX, 4, ch_b), _BF16)
    for t in range(T_STEPS):
        tg, u = divmod(t, 4)
        xt[:, tg, :, 0:I_DIM, u, :] = xf[:, t].astype(_BF16)
        xt[:, tg, :, I_DIM, u, :] = _BF16(1.0)
    m = {"xt": xt.reshape(n_sc, N_TG, KB, 4, ch_b)}
    m.update(weight_arrs)
    return m


def _prep_weights(W_ih, W_hh, b_ih, b_hh, W_fc, b_fc):
    W_ih = np.asarray(W_ih, dtype=np.float32)
    W_hh = np.asarray(W_hh, dtype=np.float32)
    b = np.asarray(b_ih, dtype=np.float32) + np.asarray(b_hh, dtype=np.float32)
    W_fc = np.asarray(W_fc, dtype=np.float32)
    b_fc = np.asarray(b_fc, dtype=np.float32)

    wx = np.zeros((KB, 512), np.float32)
    wh = np.zeros((128, 512), np.float32)
    wfc = np.zeros((128, 128), np.float32)
    for g in range(4):
        pg = GATE_PERM[g]
        rows = slice(32 * pg, 32 * pg + 32)
        for j in range(4):
            wx[KX * j:KX * j + I_DIM, 128 * g + 32 * j:128 * g + 32 * j + 32] \
                = W_ih[rows, :].T
            wx[KX * j + I_DIM, 128 * g + 32 * j:128 * g + 32 * j + 32] \
                = b[rows]
            wh[32 * j:32 * j + 32, 128 * g + 32 * j:128 * g + 32 * j + 32] \
                = W_hh[rows, :].T
    for j in range(4):
        wfc[32 * j:32 * j + 32, 32 * j:32 * j + C_DIM] = W_fc.T
    bfc = np.zeros((128, 1), np.float32)
    for j in range(4):
        bfc[32 * j:32 * j + C_DIM, 0] = b_fc
    return {
        "wx": wx.astype(_BF16),
        "wh": wh.astype(_BF16),
        "wfc": wfc.astype(_BF16),
        "bfc": bfc,
    }


def _run(inputs, trace=False):
    from concourse.bass_utils import run_bass_kernel_spmd

    nc = _get_nc()
    x = np.asarray(inputs["x"], dtype=np.float32)
    w = _prep_weights(inputs["W_ih"], inputs["W_hh"], inputs["b_ih"],
                      inputs["b_hh"], inputs["W_fc"], inputs["b_fc"])
    b_core = B_FULL // N_CORES
    in_maps = [
        _prep_core_inputs(x[i * b_core:(i + 1) * b_core], w)
        for i in range(N_CORES)
    ]
    last_err = None
    for attempt in range(4):
        try:
            res = run_bass_kernel_spmd(
                nc, in_maps, core_ids=list(range(N_CORES)), trace=trace,
            )
            break
        except Exception as e:  # transient device wedges: retry
            last_err = e
            import time as _time
            _time.sleep(3.0)
    else:
        raise last_err
    out = np.concatenate(
        [np.asarray(res.results[i]["out"]) for i in range(N_CORES)], axis=0
    )
    return out, res


def kernel(x, W_ih, W_hh, b_ih, b_hh, W_fc, b_fc):
    out, _ = _run(dict(x=x, W_ih=W_ih, W_hh=W_hh, b_ih=b_ih, b_hh=b_hh,
                       W_fc=W_fc, b_fc=b_fc))
    return out


# revision 5
# speedup vs baseline: 1.0294x; 1.0294x over previous
"""Trainium2 Bass kernel for nn_AudioRNN (LSTM(13->32, T=25) + FC(32->4), B=65536).

Strategy v2 (pure data parallel over batch, 8 cores x 8192 rows):

  * Same chunk-quadrant elementwise layout as v1: superchunk = 4 chunks of
    512 batch rows; chunk c occupies SBUF/PSUM partition quadrant c (32
    partitions = 32 hidden dims); gate pre-activations for one (t, sc) in one
    PSUM tile [128, 4*512] with free-dim bank G = gate G (f, i, o, g).
  * v2 matmuls are full-PE-array with BLOCK-DIAGONAL stationary weights so a
    single instruction covers all 4 chunks:
      - input proj, gate g: lhsT [56, 128], block j = rows 14j..14j+14 x cols
        32j..32j+32 holding [W_ih_g; b_g]^T; rhs = xt tile [56, 512] with
        chunk j's (x_t, ones) on rows 14j..14j+14.  4 instructions/(t, sc).
      - recurrence, gate g: lhsT [128, 128], block j = W_hh_g^T on the
        diagonal; rhs = h tile [128, 512] as produced by the elementwise
        stage.  4 instructions/(t, sc), accumulated into the same PSUM bank.
    => 8 matmuls x 512 free rows per (t, sc) instead of v1's 32.
  * FC: one block-diag [128, 128] matmul per superchunk (W_fc zero-padded).
"""

import numpy as np
import ml_dtypes

I_DIM = 13
H_DIM = 32
C_DIM = 4
T_STEPS = 25
B_FULL = 65536

N_TG = (T_STEPS + 3) // 4    # 7 t-groups of up to 4 timesteps
KX = I_DIM + 1               # 14: 13 input dims + ones row for bias
KB = 4 * KX                  # 56: stacked input rows for 4 chunks

# free-dim bank order of the gates: f, i, o, g  (sigmoid on banks 0..2, tanh on 3)
# -> PyTorch row-chunk order in W_ih/W_hh is i(0), f(1), g(2), o(3)
GATE_PERM = [1, 0, 3, 2]     # bank G -> pytorch gate chunk index

N_CORES = 8
CH_B = 512                   # batch rows per chunk (= one PSUM bank of fp32)
N_SC = 4                     # superchunks per core

_BF16 = ml_dtypes.bfloat16

_NC_CACHE = {}


def _build_bass(n_sc=N_SC, ch_b=CH_B, split_waits=True):
    import concourse.bass as bass
    import concourse.mybir as mybir
    from concourse.tile import TileContext

    dt = mybir.dt
    AF = mybir.ActivationFunctionType

    sc_b = 4 * ch_b
    b_core = n_sc * sc_b

    nc = bass.Bass("TRN2")

    # xt[s, tg, :, u, :]: rows 14j+r = chunk j input row r at t = 4*tg+u
    xt_d = nc.dram_tensor("xt", [n_sc, N_TG, KB, 4, ch_b], dt.bfloat16,
                          kind="ExternalInput")
    wx_d = nc.dram_tensor("wx", [KB, 512], dt.bfloat16, kind="ExternalInput")
    wh_d = nc.dram_tensor("wh", [128, 512], dt.bfloat16, kind="ExternalInput")
    wfc_d = nc.dram_tensor("wfc", [128, 128], dt.bfloat16,
                           kind="ExternalInput")
    bfc_d = nc.dram_tensor("bfc", [128, 1], dt.float32, kind="ExternalInput")
    out_d = nc.dram_tensor("out", [b_core, C_DIM], dt.float32,
                           kind="ExternalOutput")

    c_dt = dt.bfloat16  # dtype of the cell state c

    with TileContext(nc) as tc:
        with (
            tc.tile_pool(name="singles", bufs=1) as singles,
            tc.tile_pool(name="xt", bufs=n_sc * N_TG) as xt_pool,
            tc.tile_pool(name="sig", bufs=6) as sig_pool,
            tc.tile_pool(name="cell", bufs=6) as cell_pool,
            tc.tile_pool(name="tanh", bufs=4) as tanh_pool,
            tc.tile_pool(name="hid", bufs=8) as hid_pool,
            tc.tile_pool(name="tmp", bufs=6) as tmp_pool,
            tc.tile_pool(name="outp", bufs=4) as out_pool,
            tc.tile_pool(name="psum", bufs=2, space="PSUM") as psum_pool,
            tc.tile_pool(name="psumg", bufs=2, space="PSUM") as psumg_pool,
        ):
            wx = singles.tile([KB, 512], dt.bfloat16)
            wh = singles.tile([128, 512], dt.bfloat16)
            wfc = singles.tile([128, 128], dt.bfloat16)
            bfc = singles.tile([128, 1], dt.float32)
            nc.sync.dma_start(out=wx, in_=wx_d[:, :])
            nc.sync.dma_start(out=wh, in_=wh_d[:, :])
            nc.sync.dma_start(out=wfc, in_=wfc_d[:, :])
            nc.sync.dma_start(out=bfc, in_=bfc_d[:, :])

            h_prev = [None] * n_sc
            c_prev = [None] * n_sc
            xt_cur = [None] * n_sc
            cq_cur = [None] * ((n_sc + 1) // 2)
            sig_q = [None] * n_sc

            rounds = [list(range(n_sc))]
            for rnd in rounds:
              for t in range(T_STEPS):
                tg, u = divmod(t, 4)
                for s in rnd:
                    # -- stage the pre-packed x for this t-group
                    if u == 0:
                        xt = xt_pool.tile([KB, 4 * ch_b], dt.bfloat16,
                                          tag="xt")
                        eng = (nc.sync, nc.gpsimd)[s % 2]
                        eng.dma_start(
                            out=xt, in_=xt_d[s, tg].rearrange("p u b -> p (u b)"))
                        xt_cur[s] = xt
                    xr = xt_cur[s]

                    # -- gate pre-activations: a 3-bank (f,i,o) PSUM tile
                    # and a 1-bank (g) PSUM tile, each freed by its single
                    # ACT reader so the slots recycle early.
                    P = psum_pool.tile([128, 3 * ch_b], dt.float32,
                                       tag="gates")
                    Pg = psumg_pool.tile([128, ch_b], dt.float32, tag="g")
                    for g in range(4):
                        dst = Pg if g == 3 else P[:, ch_b * g:ch_b * (g + 1)]
                        nc.tensor.matmul(
                            out=dst,
                            lhsT=wx[:, 128 * g:128 * g + 128],
                            rhs=xr[:, ch_b * u:ch_b * (u + 1)],
                            start=True,
                            stop=(t == 0),
                            skip_group_check=True,
                        )
                    if t > 0:
                        for g in range(4):
                            dst = Pg if g == 3 else P[:, ch_b * g:ch_b * (g + 1)]
                            nc.tensor.matmul(
                                out=dst,
                                lhsT=wh[:, 128 * g:128 * g + 128],
                                rhs=h_prev[s],
                                start=False,
                                stop=True,
                                skip_group_check=True,
                            )

                    # -- activations: sigmoid(f,i,o) in one op, tanh(g)
                    S = sig_pool.tile([128, 3 * ch_b], dt.bfloat16, tag="S")
                    nc.scalar.activation(out=S, in_=P[:, 0:3 * ch_b],
                                         func=AF.Sigmoid)
                    Gt = tmp_pool.tile([128, ch_b], dt.bfloat16, tag="Gt")
                    nc.scalar.activation(out=Gt, in_=Pg,
                                         func=AF.Tanh)

                    # -- cell update (all lane-aligned, 128 partitions
                    # busy).  c for all n_sc superchunks lives in ONE shared
                    # tile so tanh(c) batches into a single ACT instruction
                    # per timestep.
                    if s % 2 == 0:
                        Cq = cell_pool.tile([128, 2 * ch_b], c_dt,
                                            tag="C")
                        cq_cur[s // 2] = Cq
                    else:
                        Cq = cq_cur[s // 2]
                    Cn = Cq[:, (s % 2) * ch_b:(s % 2 + 1) * ch_b]
                    if t == 0:
                        nc.vector.tensor_mul(Cn, S[:, ch_b:2 * ch_b], Gt)
                    else:
                        FCt = tmp_pool.tile([128, ch_b], c_dt, tag="FCt")
                        IGt = tmp_pool.tile([128, ch_b], c_dt, tag="IGt")
                        nc.vector.tensor_mul(FCt, S[:, 0:ch_b], c_prev[s])
                        nc.vector.tensor_mul(IGt, S[:, ch_b:2 * ch_b], Gt)
                        nc.vector.tensor_add(Cn, FCt, IGt)
                    c_prev[s] = Cn
                    sig_q[s] = S

                    if s % 2 == 1:
                        Tc = tanh_pool.tile([128, 2 * ch_b], dt.bfloat16,
                                            tag="Tc")
                        nc.scalar.activation(out=Tc, in_=Cq, func=AF.Tanh)
                        for sp in (s - 1, s):
                            Sp = sig_q[sp]
                            Hn = hid_pool.tile([128, ch_b], dt.bfloat16,
                                               tag="H")
                            nc.vector.tensor_mul(
                                Hn, Sp[:, 2 * ch_b:3 * ch_b],
                                Tc[:, (sp % 2) * ch_b:(sp % 2 + 1) * ch_b])
                            h_prev[sp] = Hn

                            # -- final FC + bias + store, per superchunk
                            if t == T_STEPS - 1:
                                PF = psumg_pool.tile([128, ch_b], dt.float32,
                                                     tag="g")
                                nc.tensor.matmul(
                                    out=PF,
                                    lhsT=wfc,
                                    rhs=Hn,
                                    start=True,
                                    stop=True,
                                    skip_group_check=True,
                                )
                                Ot = out_pool.tile([128, ch_b], dt.float32,
                                                   tag="O")
                                nc.scalar.add(Ot, PF, bfc)
                                for c in range(4):
                                    r0 = sp * sc_b + c * ch_b
                                    dst = out_d[r0:r0 + ch_b, :].rearrange(
                                        "b m -> m b")
                                    eng = (nc.gpsimd, nc.sync,
                                           nc.gpsimd, nc.sync)[c]
                                    eng.dma_start(
                                        out=dst,
                                        in_=Ot[32 * c:32 * c + C_DIM, :])

    if split_waits:
        _split_multi_waits(nc, mybir)
    return nc


def _split_multi_waits(nc, mybir):
    """This walrus build allows only ONE sync-wait command per ISA
    instruction.  Tile sometimes emits 2+ (its wait minimization is not
    transitive across processors).  Hoist all-but-one wait onto standalone
    EventSemaphore instructions injected just before, on the same engine —
    semantically identical (the engine stream blocks at the wait either way).
    """
    n_split = 0
    for fn in nc.m.functions:
        for blk in fn.blocks:
            out = []
            for inst in blk.instructions:
                si = getattr(inst, "sync_info", None)
                ow = list(si.on_wait) if si is not None and si.on_wait else []
                if len(ow) > 1 and inst.opcode == "DMACopy" \
                        and str(inst.engine) in ("EngineType.SP",
                                                 "EngineType.Activation"):
                    # Keep the HWDGE queue-slot wait on the DMA descriptor;
                    # hoist data-dependency waits onto the engine stream
                    # (SP blocks before issuing the descriptor - a strictly
                    # stronger ordering, so semantically safe).
                    qw = [w for w in ow if "DMA" in (w.ant_name or "")]
                    rest = [w for w in ow if "DMA" not in (w.ant_name or "")]
                    ow = rest + (qw[-1:] if qw else rest[-1:])
                    ow = rest + qw[-1:] if qw else rest
                if len(ow) > 1:
                    for w in ow[:-1]:
                        n_split += 1
                        ev = mybir.InstEventSemaphore(
                            name=f"splitw-{n_split}-{inst.name}",
                            engine=inst.engine,
                            ins=[],
                            outs=[],
                            sync_info=mybir.SyncInfo(on_wait=[w],
                                                     on_update=[]),
                            bass_priority=inst.bass_priority,
                            bass_scheduled_tick=inst.bass_scheduled_tick,
                            bass_scheduled_proc=inst.bass_scheduled_proc,
                            bass_scheduled_scope=inst.bass_scheduled_scope,
                        )
                        nc.inst_map[ev.name] = ev
                        out.append(ev)
                    si.on_wait = ow[-1:]
                out.append(inst)
            blk.instructions = out
    return n_split


def _get_nc():
    if "nc" not in _NC_CACHE:
        _NC_CACHE["nc"] = _build_bass()
    return _NC_CACHE["nc"]


def _prep_core_inputs(x_core, weight_arrs, n_sc=N_SC, ch_b=CH_B):
    """x_core: [b_core, T, I] fp32 -> the per-core input map."""
    # [sc, chunk j, b, t, i]
    xr = x_core.reshape(n_sc, 4, ch_b, T_STEPS, I_DIM)
    # -> [sc, t, j, i, b]
    xf = np.ascontiguousarray(xr.transpose(0, 3, 1, 4, 2))
    xt = np.zeros((n_sc, N_TG, 4, KX, 4, ch_b), _BF16)
    for t in range(T_STEPS):
        tg, u = divmod(t, 4)
        xt[:, tg, :, 0:I_DIM, u, :] = xf[:, t].astype(_BF16)
        xt[:, tg, :, I_DIM, u, :] = _BF16(1.0)
    m = {"xt": xt.reshape(n_sc, N_TG, KB, 4, ch_b)}
    m.update(weight_arrs)
    return m


def _prep_weights(W_ih, W_hh, b_ih, b_hh, W_fc, b_fc):
    W_ih = np.asarray(W_ih, dtype=np.float32)
    W_hh = np.asarray(W_hh, dtype=np.float32)
    b = np.asarray(b_ih, dtype=np.float32) + np.asarray(b_hh, dtype=np.float32)
    W_fc = np.asarray(W_fc, dtype=np.float32)
    b_fc = np.asarray(b_fc, dtype=np.float32)

    wx = np.zeros((KB, 512), np.float32)
    wh = np.zeros((128, 512), np.float32)
    wfc = np.zeros((128, 128), np.float32)
    for g in range(4):
        pg = GATE_PERM[g]
        rows = slice(32 * pg, 32 * pg + 32)
        for j in range(4):
            wx[KX * j:KX * j + I_DIM, 128 * g + 32 * j:128 * g + 32 * j + 32] \
                = W_ih[rows, :].T
            wx[KX * j + I_DIM, 128 * g + 32 * j:128 * g + 32 * j + 32] \
                = b[rows]
            wh[32 * j:32 * j + 32, 128 * g + 32 * j:128 * g + 32 * j + 32] \
                = W_hh[rows, :].T
    for j in range(4):
        wfc[32 * j:32 * j + 32, 32 * j:32 * j + C_DIM] = W_fc.T
    bfc = np.zeros((128, 1), np.float32)
    for j in range(4):
        bfc[32 * j:32 * j + C_DIM, 0] = b_fc
    return {
        "wx": wx.astype(_BF16),
        "wh": wh.astype(_BF16),
        "wfc": wfc.astype(_BF16),
        "bfc": bfc,
    }


def _run(inputs, trace=False):
    from concourse.bass_utils import run_bass_kernel_spmd

    nc = _get_nc()
    x = np.asarray(inputs["x"], dtype=np.float32)
    w = _prep_weights(inputs["W_ih"], inputs["W_hh"], inputs["b_ih"],
                      inputs["b_hh"], inputs["W_fc"], inputs["b_fc"])
    b_core = B_FULL // N_CORES
    in_maps = [
        _prep_core_inputs(x[i * b_core:(i + 1) * b_core], w)
        for i in range(N_CORES)
    ]
    last_err = None
    for attempt in range(4):
        try:
            res = run_bass_kernel_spmd(
                nc, in_maps, core_ids=list(range(N_CORES)), trace=trace,
            )
            break
        except Exception as e:  # transient device wedges: retry
            last_err = e
            import time as _time
            _time.sleep(3.0)
    else:
        raise last_err
    out = np.concatenate(
        [np.asarray(res.results[i]["out"]) for i in range(N_CORES)], axis=0
    )
    return out, res


def kernel(x, W_ih, W_hh, b_ih, b_hh, W_fc, b_fc):
    out, _ = _run(dict(x=x, W_ih=W_ih, W_hh=W_hh, b_ih=b_ih, b_hh=b_hh,
                       W_fc=W_fc, b_fc=b_fc))
    return out


# revision 6
# speedup vs baseline: 1.0849x; 1.0539x over previous
"""Trainium2 Bass kernel for nn_AudioRNN (LSTM(13->32, T=25) + FC(32->4), B=65536).

Strategy v2 (pure data parallel over batch, 8 cores x 8192 rows):

  * Same chunk-quadrant elementwise layout as v1: superchunk = 4 chunks of
    512 batch rows; chunk c occupies SBUF/PSUM partition quadrant c (32
    partitions = 32 hidden dims); gate pre-activations for one (t, sc) in one
    PSUM tile [128, 4*512] with free-dim bank G = gate G (f, i, o, g).
  * v2 matmuls are full-PE-array with BLOCK-DIAGONAL stationary weights so a
    single instruction covers all 4 chunks:
      - input proj, gate g: lhsT [56, 128], block j = rows 14j..14j+14 x cols
        32j..32j+32 holding [W_ih_g; b_g]^T; rhs = xt tile [56, 512] with
        chunk j's (x_t, ones) on rows 14j..14j+14.  4 instructions/(t, sc).
      - recurrence, gate g: lhsT [128, 128], block j = W_hh_g^T on the
        diagonal; rhs = h tile [128, 512] as produced by the elementwise
        stage.  4 instructions/(t, sc), accumulated into the same PSUM bank.
    => 8 matmuls x 512 free rows per (t, sc) instead of v1's 32.
  * FC: one block-diag [128, 128] matmul per superchunk (W_fc zero-padded).
"""

import numpy as np
import ml_dtypes

I_DIM = 13
H_DIM = 32
C_DIM = 4
T_STEPS = 25
B_FULL = 65536

N_TG = (T_STEPS + 3) // 4    # 7 t-groups of up to 4 timesteps
KX = I_DIM + 1               # 14: 13 input dims + ones row for bias
KB = 4 * KX                  # 56: stacked input rows for 4 chunks

# free-dim bank order of the gates: f, i, o, g  (sigmoid on banks 0..2, tanh on 3)
# -> PyTorch row-chunk order in W_ih/W_hh is i(0), f(1), g(2), o(3)
GATE_PERM = [1, 0, 3, 2]     # bank G -> pytorch gate chunk index

N_CORES = 8
CH_B = 512                   # batch rows per chunk (= one PSUM bank of fp32)
N_SC = 4                     # superchunks per core

_BF16 = ml_dtypes.bfloat16

_NC_CACHE = {}


def _build_bass(n_sc=N_SC, ch_b=CH_B, split_waits=True):
    import concourse.bass as bass
    import concourse.mybir as mybir
    from concourse.tile import TileContext

    dt = mybir.dt
    AF = mybir.ActivationFunctionType

    sc_b = 4 * ch_b
    b_core = n_sc * sc_b

    nc = bass.Bass("TRN2")

    # xt[s, tg, :, u, :]: rows 14j+r = chunk j input row r at t = 4*tg+u
    xt_d = nc.dram_tensor("xt", [n_sc, N_TG, KB, 4, ch_b], dt.bfloat16,
                          kind="ExternalInput")
    wx_d = nc.dram_tensor("wx", [KB, 512], dt.bfloat16, kind="ExternalInput")
    wh_d = nc.dram_tensor("wh", [128, 512], dt.bfloat16, kind="ExternalInput")
    wfc_d = nc.dram_tensor("wfc", [128, 128], dt.bfloat16,
                           kind="ExternalInput")
    bfc_d = nc.dram_tensor("bfc", [128, 1], dt.float32, kind="ExternalInput")
    out_d = nc.dram_tensor("out", [b_core, C_DIM], dt.float32,
                           kind="ExternalOutput")

    c_dt = dt.bfloat16  # dtype of the cell state c

    with TileContext(nc) as tc:
        with (
            tc.tile_pool(name="singles", bufs=1) as singles,
            tc.tile_pool(name="xt", bufs=n_sc * N_TG) as xt_pool,
            tc.tile_pool(name="sig", bufs=6) as sig_pool,
            tc.tile_pool(name="cell", bufs=6) as cell_pool,
            tc.tile_pool(name="tanh", bufs=4) as tanh_pool,
            tc.tile_pool(name="hid", bufs=8) as hid_pool,
            tc.tile_pool(name="tmp", bufs=6) as tmp_pool,
            tc.tile_pool(name="outp", bufs=4) as out_pool,
            tc.tile_pool(name="psum", bufs=2, space="PSUM") as psum_pool,
        ):
            wx = singles.tile([KB, 512], dt.bfloat16)
            wh = singles.tile([128, 512], dt.bfloat16)
            wfc = singles.tile([128, 128], dt.bfloat16)
            bfc = singles.tile([128, 1], dt.float32)
            nc.sync.dma_start(out=wx, in_=wx_d[:, :])
            nc.sync.dma_start(out=wh, in_=wh_d[:, :])
            nc.sync.dma_start(out=wfc, in_=wfc_d[:, :])
            nc.sync.dma_start(out=bfc, in_=bfc_d[:, :])

            h_prev = [None] * n_sc
            c_prev = [None] * n_sc
            xt_cur = [None] * n_sc
            cq_cur = [None] * ((n_sc + 1) // 2)
            sig_q = [None] * n_sc

            rounds = [list(range(n_sc))]
            for rnd in rounds:
              for t in range(T_STEPS):
                tg, u = divmod(t, 4)
                for s in rnd:
                    # -- stage the pre-packed x for this t-group
                    if u == 0:
                        xt = xt_pool.tile([KB, 4 * ch_b], dt.bfloat16,
                                          tag="xt")
                        eng = (nc.sync, nc.gpsimd)[s % 2]
                        eng.dma_start(
                            out=xt, in_=xt_d[s, tg].rearrange("p u b -> p (u b)"))
                        xt_cur[s] = xt
                    xr = xt_cur[s]

                    # -- gate pre-activations: one 4-bank PSUM tile.  The
                    # g-gate weights/bias are pre-doubled on the host, so a
                    # SINGLE sigmoid covers all four banks (tanh(z) =
                    # 2*sigmoid(2z)-1; the affine fixup is one fused DVE
                    # tensor_scalar op at 4x rate).
                    P = psum_pool.tile([128, 4 * ch_b], dt.float32,
                                       tag="gates")
                    for g in range(4):
                        nc.tensor.matmul(
                            out=P[:, ch_b * g:ch_b * (g + 1)],
                            lhsT=wx[:, 128 * g:128 * g + 128],
                            rhs=xr[:, ch_b * u:ch_b * (u + 1)],
                            start=True,
                            stop=(t == 0),
                            skip_group_check=True,
                        )
                    if t > 0:
                        for g in range(4):
                            nc.tensor.matmul(
                                out=P[:, ch_b * g:ch_b * (g + 1)],
                                lhsT=wh[:, 128 * g:128 * g + 128],
                                rhs=h_prev[s],
                                start=False,
                                stop=True,
                                skip_group_check=True,
                            )

                    # -- activations: ONE sigmoid over (f, i, o, g2)
                    S = sig_pool.tile([128, 4 * ch_b], dt.bfloat16, tag="S")
                    nc.scalar.activation(out=S, in_=P,
                                         func=AF.Sigmoid)
                    Gt = tmp_pool.tile([128, ch_b], dt.bfloat16, tag="Gt")
                    nc.vector.tensor_scalar(
                        out=Gt, in0=S[:, 3 * ch_b:4 * ch_b], scalar1=2.0,
                        scalar2=-1.0, op0=mybir.AluOpType.mult,
                        op1=mybir.AluOpType.add)

                    # -- cell update (all lane-aligned, 128 partitions
                    # busy).  c for all n_sc superchunks lives in ONE shared
                    # tile so tanh(c) batches into a single ACT instruction
                    # per timestep.
                    if s % 2 == 0:
                        Cq = cell_pool.tile([128, 2 * ch_b], c_dt,
                                            tag="C")
                        cq_cur[s // 2] = Cq
                    else:
                        Cq = cq_cur[s // 2]
                    Cn = Cq[:, (s % 2) * ch_b:(s % 2 + 1) * ch_b]
                    if t == 0:
                        nc.vector.tensor_mul(Cn, S[:, ch_b:2 * ch_b], Gt)
                    else:
                        FCt = tmp_pool.tile([128, ch_b], c_dt, tag="FCt")
                        IGt = tmp_pool.tile([128, ch_b], c_dt, tag="IGt")
                        nc.vector.tensor_mul(FCt, S[:, 0:ch_b], c_prev[s])
                        nc.vector.tensor_mul(IGt, S[:, ch_b:2 * ch_b], Gt)
                        nc.vector.tensor_add(Cn, FCt, IGt)
                    c_prev[s] = Cn
                    sig_q[s] = S

                    if s % 2 == 1:
                        Tc = tanh_pool.tile([128, 2 * ch_b], dt.bfloat16,
                                            tag="Tc")
                        nc.scalar.activation(out=Tc, in_=Cq, func=AF.Tanh)
                        for sp in (s - 1, s):
                            Sp = sig_q[sp]
                            Hn = hid_pool.tile([128, ch_b], dt.bfloat16,
                                               tag="H")
                            nc.vector.tensor_mul(
                                Hn, Sp[:, 2 * ch_b:3 * ch_b],
                                Tc[:, (sp % 2) * ch_b:(sp % 2 + 1) * ch_b])
                            h_prev[sp] = Hn

                            # -- final FC + bias + store, per superchunk
                            if t == T_STEPS - 1:
                                PF = psum_pool.tile([128, ch_b], dt.float32,
                                                    tag="gates")
                                nc.tensor.matmul(
                                    out=PF,
                                    lhsT=wfc,
                                    rhs=Hn,
                                    start=True,
                                    stop=True,
                                    skip_group_check=True,
                                )
                                Ot = out_pool.tile([128, ch_b], dt.float32,
                                                   tag="O")
                                nc.scalar.add(Ot, PF, bfc)
                                for c in range(4):
                                    r0 = sp * sc_b + c * ch_b
                                    dst = out_d[r0:r0 + ch_b, :].rearrange(
                                        "b m -> m b")
                                    eng = (nc.gpsimd, nc.sync,
                                           nc.gpsimd, nc.sync)[c]
                                    eng.dma_start(
                                        out=dst,
                                        in_=Ot[32 * c:32 * c + C_DIM, :])

    if split_waits:
        _split_multi_waits(nc, mybir)
    return nc


def _split_multi_waits(nc, mybir):
    """This walrus build allows only ONE sync-wait command per ISA
    instruction.  Tile sometimes emits 2+ (its wait minimization is not
    transitive across processors).  Hoist all-but-one wait onto standalone
    EventSemaphore instructions injected just before, on the same engine —
    semantically identical (the engine stream blocks at the wait either way).
    """
    n_split = 0
    for fn in nc.m.functions:
        for blk in fn.blocks:
            out = []
            for inst in blk.instructions:
                si = getattr(inst, "sync_info", None)
                ow = list(si.on_wait) if si is not None and si.on_wait else []
                if len(ow) > 1 and inst.opcode == "DMACopy" \
                        and str(inst.engine) in ("EngineType.SP",
                                                 "EngineType.Activation"):
                    # Keep the HWDGE queue-slot wait on the DMA descriptor;
                    # hoist data-dependency waits onto the engine stream
                    # (SP blocks before issuing the descriptor - a strictly
                    # stronger ordering, so semantically safe).
                    qw = [w for w in ow if "DMA" in (w.ant_name or "")]
                    rest = [w for w in ow if "DMA" not in (w.ant_name or "")]
                    ow = rest + (qw[-1:] if qw else rest[-1:])
                    ow = rest + qw[-1:] if qw else rest
                if len(ow) > 1:
                    for w in ow[:-1]:
                        n_split += 1
                        ev = mybir.InstEventSemaphore(
                            name=f"splitw-{n_split}-{inst.name}",
                            engine=inst.engine,
                            ins=[],
                            outs=[],
                            sync_info=mybir.SyncInfo(on_wait=[w],
                                                     on_update=[]),
                            bass_priority=inst.bass_priority,
                            bass_scheduled_tick=inst.bass_scheduled_tick,
                            bass_scheduled_proc=inst.bass_scheduled_proc,
                            bass_scheduled_scope=inst.bass_scheduled_scope,
                        )
                        nc.inst_map[ev.name] = ev
                        out.append(ev)
                    si.on_wait = ow[-1:]
                out.append(inst)
            blk.instructions = out
    return n_split


def _get_nc():
    if "nc" not in _NC_CACHE:
        _NC_CACHE["nc"] = _build_bass()
    return _NC_CACHE["nc"]


def _prep_core_inputs(x_core, weight_arrs, n_sc=N_SC, ch_b=CH_B):
    """x_core: [b_core, T, I] fp32 -> the per-core input map."""
    # [sc, chunk j, b, t, i]
    xr = x_core.reshape(n_sc, 4, ch_b, T_STEPS, I_DIM)
    # -> [sc, t, j, i, b]
    xf = np.ascontiguousarray(xr.transpose(0, 3, 1, 4, 2))
    xt = np.zeros((n_sc, N_TG, 4, KX, 4, ch_b), _BF16)
    for t in range(T_STEPS):
        tg, u = divmod(t, 4)
        xt[:, tg, :, 0:I_DIM, u, :] = xf[:, t].astype(_BF16)
        xt[:, tg, :, I_DIM, u, :] = _BF16(1.0)
    m = {"xt": xt.reshape(n_sc, N_TG, KB, 4, ch_b)}
    m.update(weight_arrs)
    return m


def _prep_weights(W_ih, W_hh, b_ih, b_hh, W_fc, b_fc):
    W_ih = np.asarray(W_ih, dtype=np.float32)
    W_hh = np.asarray(W_hh, dtype=np.float32)
    b = np.asarray(b_ih, dtype=np.float32) + np.asarray(b_hh, dtype=np.float32)
    W_fc = np.asarray(W_fc, dtype=np.float32)
    b_fc = np.asarray(b_fc, dtype=np.float32)

    wx = np.zeros((KB, 512), np.float32)
    wh = np.zeros((128, 512), np.float32)
    wfc = np.zeros((128, 128), np.float32)
    for g in range(4):
        pg = GATE_PERM[g]
        rows = slice(32 * pg, 32 * pg + 32)
        gs = 2.0 if g == 3 else 1.0  # tanh-as-sigmoid: double the g bank
        for j in range(4):
            wx[KX * j:KX * j + I_DIM, 128 * g + 32 * j:128 * g + 32 * j + 32] \
                = gs * W_ih[rows, :].T
            wx[KX * j + I_DIM, 128 * g + 32 * j:128 * g + 32 * j + 32] \
                = gs * b[rows]
            wh[32 * j:32 * j + 32, 128 * g + 32 * j:128 * g + 32 * j + 32] \
                = gs * W_hh[rows, :].T
    for j in range(4):
        wfc[32 * j:32 * j + 32, 32 * j:32 * j + C_DIM] = W_fc.T
    bfc = np.zeros((128, 1), np.float32)
    for j in range(4):
        bfc[32 * j:32 * j + C_DIM, 0] = b_fc
    return {
        "wx": wx.astype(_BF16),
        "wh": wh.astype(_BF16),
        "wfc": wfc.astype(_BF16),
        "bfc": bfc,
    }


def _run(inputs, trace=False):
    from concourse.bass_utils import run_bass_kernel_spmd

    nc = _get_nc()
    x = np.asarray(inputs["x"], dtype=np.float32)
    w = _prep_weights(inputs["W_ih"], inputs["W_hh"], inputs["b_ih"],
                      inputs["b_hh"], inputs["W_fc"], inputs["b_fc"])
    b_core = B_FULL // N_CORES
    in_maps = [
        _prep_core_inputs(x[i * b_core:(i + 1) * b_core], w)
        for i in range(N_CORES)
    ]
    last_err = None
    for attempt in range(4):
        try:
            res = run_bass_kernel_spmd(
                nc, in_maps, core_ids=list(range(N_CORES)), trace=trace,
            )
            break
        except Exception as e:  # transient device wedges: retry
            last_err = e
            import time as _time
            _time.sleep(3.0)
    else:
        raise last_err
    out = np.concatenate(
        [np.asarray(res.results[i]["out"]) for i in range(N_CORES)], axis=0
    )
    return out, res


def kernel(x, W_ih, W_hh, b_ih, b_hh, W_fc, b_fc):
    out, _ = _run(dict(x=x, W_ih=W_ih, W_hh=W_hh, b_ih=b_ih, b_hh=b_hh,
                       W_fc=W_fc, b_fc=b_fc))
    return out


# revision 7
# speedup vs baseline: 1.0935x; 1.0080x over previous
"""Trainium2 Bass kernel for nn_AudioRNN (LSTM(13->32, T=25) + FC(32->4), B=65536).

Strategy v2 (pure data parallel over batch, 8 cores x 8192 rows):

  * Same chunk-quadrant elementwise layout as v1: superchunk = 4 chunks of
    512 batch rows; chunk c occupies SBUF/PSUM partition quadrant c (32
    partitions = 32 hidden dims); gate pre-activations for one (t, sc) in one
    PSUM tile [128, 4*512] with free-dim bank G = gate G (f, i, o, g).
  * v2 matmuls are full-PE-array with BLOCK-DIAGONAL stationary weights so a
    single instruction covers all 4 chunks:
      - input proj, gate g: lhsT [56, 128], block j = rows 14j..14j+14 x cols
        32j..32j+32 holding [W_ih_g; b_g]^T; rhs = xt tile [56, 512] with
        chunk j's (x_t, ones) on rows 14j..14j+14.  4 instructions/(t, sc).
      - recurrence, gate g: lhsT [128, 128], block j = W_hh_g^T on the
        diagonal; rhs = h tile [128, 512] as produced by the elementwise
        stage.  4 instructions/(t, sc), accumulated into the same PSUM bank.
    => 8 matmuls x 512 free rows per (t, sc) instead of v1's 32.
  * FC: one block-diag [128, 128] matmul per superchunk (W_fc zero-padded).
"""

import numpy as np
import ml_dtypes

I_DIM = 13
H_DIM = 32
C_DIM = 4
T_STEPS = 25
B_FULL = 65536

N_TG = (T_STEPS + 3) // 4    # 7 t-groups of up to 4 timesteps
KX = I_DIM + 1               # 14: 13 input dims + ones row for bias
KB = 4 * KX                  # 56: stacked input rows for 4 chunks

# free-dim bank order of the gates: f, i, o, g  (sigmoid on banks 0..2, tanh on 3)
# -> PyTorch row-chunk order in W_ih/W_hh is i(0), f(1), g(2), o(3)
GATE_PERM = [1, 0, 3, 2]     # bank G -> pytorch gate chunk index

N_CORES = 8
CH_B = 512                   # batch rows per chunk (= one PSUM bank of fp32)
N_SC = 4                     # superchunks per core

_BF16 = ml_dtypes.bfloat16

_NC_CACHE = {}


def _build_bass(n_sc=N_SC, ch_b=CH_B, split_waits=True):
    import concourse.bass as bass
    import concourse.mybir as mybir
    from concourse.tile import TileContext

    dt = mybir.dt
    AF = mybir.ActivationFunctionType

    sc_b = 4 * ch_b
    b_core = n_sc * sc_b

    nc = bass.Bass("TRN2")

    # xt[s, tg, :, u, :]: rows 14j+r = chunk j input row r at t = 4*tg+u
    xt_d = nc.dram_tensor("xt", [n_sc, N_TG, KB, 4, ch_b], dt.bfloat16,
                          kind="ExternalInput")
    wx_d = nc.dram_tensor("wx", [KB, 512], dt.bfloat16, kind="ExternalInput")
    wh_d = nc.dram_tensor("wh", [128, 512], dt.bfloat16, kind="ExternalInput")
    wfc_d = nc.dram_tensor("wfc", [128, 128], dt.bfloat16,
                           kind="ExternalInput")
    bfc_d = nc.dram_tensor("bfc", [128, 1], dt.float32, kind="ExternalInput")
    out_d = nc.dram_tensor("out", [b_core, C_DIM], dt.float32,
                           kind="ExternalOutput")

    c_dt = dt.bfloat16  # dtype of the cell state c

    with TileContext(nc) as tc:
        with (
            tc.tile_pool(name="singles", bufs=1) as singles,
            tc.tile_pool(name="xt", bufs=n_sc * N_TG) as xt_pool,
            tc.tile_pool(name="sig", bufs=6) as sig_pool,
            tc.tile_pool(name="cell", bufs=6) as cell_pool,
            tc.tile_pool(name="tanh", bufs=4) as tanh_pool,
            tc.tile_pool(name="hid", bufs=8) as hid_pool,
            tc.tile_pool(name="tmp", bufs=6) as tmp_pool,
            tc.tile_pool(name="outp", bufs=4) as out_pool,
            tc.tile_pool(name="psum", bufs=2, space="PSUM") as psum_pool,
        ):
            wx = singles.tile([KB, 512], dt.bfloat16)
            wh = singles.tile([128, 512], dt.bfloat16)
            wfc = singles.tile([128, 128], dt.bfloat16)
            bfc = singles.tile([128, 1], dt.float32)
            # wx/wh gate the first matmuls: load them on the (otherwise
            # idle) SWDGE queue so the sync queue starts on xt immediately;
            # wfc/bfc are only needed at the end.
            nc.gpsimd.dma_start(out=wx, in_=wx_d[:, :])
            nc.gpsimd.dma_start(out=wh, in_=wh_d[:, :])
            nc.scalar.dma_start(out=wfc, in_=wfc_d[:, :])
            nc.scalar.dma_start(out=bfc, in_=bfc_d[:, :])

            h_prev = [None] * n_sc
            c_prev = [None] * n_sc
            xt_cur = [None] * n_sc
            cq_cur = [None] * ((n_sc + 1) // 2)
            sig_q = [None] * n_sc

            rounds = [list(range(n_sc))]
            for rnd in rounds:
              for t in range(T_STEPS):
                tg, u = divmod(t, 4)
                for s in rnd:
                    # -- stage the pre-packed x for this t-group
                    if u == 0:
                        xt = xt_pool.tile([KB, 4 * ch_b], dt.bfloat16,
                                          tag="xt")
                        eng = (nc.sync, nc.gpsimd)[s % 2]
                        eng.dma_start(
                            out=xt, in_=xt_d[s, tg].rearrange("p u b -> p (u b)"))
                        xt_cur[s] = xt
                    xr = xt_cur[s]

                    # -- gate pre-activations: one 4-bank PSUM tile.  The
                    # g-gate weights/bias are pre-doubled on the host, so a
                    # SINGLE sigmoid covers all four banks (tanh(z) =
                    # 2*sigmoid(2z)-1; the affine fixup is one fused DVE
                    # tensor_scalar op at 4x rate).
                    P = psum_pool.tile([128, 4 * ch_b], dt.float32,
                                       tag="gates")
                    for g in range(4):
                        nc.tensor.matmul(
                            out=P[:, ch_b * g:ch_b * (g + 1)],
                            lhsT=wx[:, 128 * g:128 * g + 128],
                            rhs=xr[:, ch_b * u:ch_b * (u + 1)],
                            start=True,
                            stop=(t == 0),
                            skip_group_check=True,
                        )
                    if t > 0:
                        for g in range(4):
                            nc.tensor.matmul(
                                out=P[:, ch_b * g:ch_b * (g + 1)],
                                lhsT=wh[:, 128 * g:128 * g + 128],
                                rhs=h_prev[s],
                                start=False,
                                stop=True,
                                skip_group_check=True,
                            )

                    # -- activations: ONE sigmoid over (f, i, o, g2)
                    S = sig_pool.tile([128, 4 * ch_b], dt.bfloat16, tag="S")
                    nc.scalar.activation(out=S, in_=P,
                                         func=AF.Sigmoid)
                    Gt = tmp_pool.tile([128, ch_b], dt.bfloat16, tag="Gt")
                    nc.vector.tensor_scalar(
                        out=Gt, in0=S[:, 3 * ch_b:4 * ch_b], scalar1=2.0,
                        scalar2=-1.0, op0=mybir.AluOpType.mult,
                        op1=mybir.AluOpType.add)

                    # -- cell update (all lane-aligned, 128 partitions
                    # busy).  c for all n_sc superchunks lives in ONE shared
                    # tile so tanh(c) batches into a single ACT instruction
                    # per timestep.
                    if s % 2 == 0:
                        Cq = cell_pool.tile([128, 2 * ch_b], c_dt,
                                            tag="C")
                        cq_cur[s // 2] = Cq
                    else:
                        Cq = cq_cur[s // 2]
                    Cn = Cq[:, (s % 2) * ch_b:(s % 2 + 1) * ch_b]
                    if t == 0:
                        nc.vector.tensor_mul(Cn, S[:, ch_b:2 * ch_b], Gt)
                    else:
                        FCt = tmp_pool.tile([128, ch_b], c_dt, tag="FCt")
                        IGt = tmp_pool.tile([128, ch_b], c_dt, tag="IGt")
                        nc.vector.tensor_mul(FCt, S[:, 0:ch_b], c_prev[s])
                        nc.vector.tensor_mul(IGt, S[:, ch_b:2 * ch_b], Gt)
                        nc.vector.tensor_add(Cn, FCt, IGt)
                    c_prev[s] = Cn
                    sig_q[s] = S

                    if s % 2 == 1:
                        Tc = tanh_pool.tile([128, 2 * ch_b], dt.bfloat16,
                                            tag="Tc")
                        nc.scalar.activation(out=Tc, in_=Cq, func=AF.Tanh)
                        for sp in (s - 1, s):
                            Sp = sig_q[sp]
                            Hn = hid_pool.tile([128, ch_b], dt.bfloat16,
                                               tag="H")
                            nc.vector.tensor_mul(
                                Hn, Sp[:, 2 * ch_b:3 * ch_b],
                                Tc[:, (sp % 2) * ch_b:(sp % 2 + 1) * ch_b])
                            h_prev[sp] = Hn

                            # -- final FC + bias + store, per superchunk
                            if t == T_STEPS - 1:
                                PF = psum_pool.tile([128, ch_b], dt.float32,
                                                    tag="gates")
                                nc.tensor.matmul(
                                    out=PF,
                                    lhsT=wfc,
                                    rhs=Hn,
                                    start=True,
                                    stop=True,
                                    skip_group_check=True,
                                )
                                Ot = out_pool.tile([128, ch_b], dt.float32,
                                                   tag="O")
                                nc.scalar.add(Ot, PF, bfc)
                                for c in range(4):
                                    r0 = sp * sc_b + c * ch_b
                                    dst = out_d[r0:r0 + ch_b, :].rearrange(
                                        "b m -> m b")
                                    eng = (nc.gpsimd, nc.sync,
                                           nc.scalar, nc.sync)[c]
                                    eng.dma_start(
                                        out=dst,
                                        in_=Ot[32 * c:32 * c + C_DIM, :])

    if split_waits:
        _split_multi_waits(nc, mybir)
    return nc


def _split_multi_waits(nc, mybir):
    """This walrus build allows only ONE sync-wait command per ISA
    instruction.  Tile sometimes emits 2+ (its wait minimization is not
    transitive across processors).  Hoist all-but-one wait onto standalone
    EventSemaphore instructions injected just before, on the same engine —
    semantically identical (the engine stream blocks at the wait either way).
    """
    n_split = 0
    for fn in nc.m.functions:
        for blk in fn.blocks:
            out = []
            for inst in blk.instructions:
                si = getattr(inst, "sync_info", None)
                ow = list(si.on_wait) if si is not None and si.on_wait else []
                if len(ow) > 1 and inst.opcode == "DMACopy" \
                        and str(inst.engine) in ("EngineType.SP",
                                                 "EngineType.Activation"):
                    # Keep the HWDGE queue-slot wait on the DMA descriptor;
                    # hoist data-dependency waits onto the engine stream
                    # (SP blocks before issuing the descriptor - a strictly
                    # stronger ordering, so semantically safe).
                    qw = [w for w in ow if "DMA" in (w.ant_name or "")]
                    rest = [w for w in ow if "DMA" not in (w.ant_name or "")]
                    ow = rest + (qw[-1:] if qw else rest[-1:])
                    ow = rest + qw[-1:] if qw else rest
                if len(ow) > 1:
                    for w in ow[:-1]:
                        n_split += 1
                        ev = mybir.InstEventSemaphore(
                            name=f"splitw-{n_split}-{inst.name}",
                            engine=inst.engine,
                            ins=[],
                            outs=[],
                            sync_info=mybir.SyncInfo(on_wait=[w],
                                                     on_update=[]),
                            bass_priority=inst.bass_priority,
                            bass_scheduled_tick=inst.bass_scheduled_tick,
                            bass_scheduled_proc=inst.bass_scheduled_proc,
                            bass_scheduled_scope=inst.bass_scheduled_scope,
                        )
                        nc.inst_map[ev.name] = ev
                        out.append(ev)
                    si.on_wait = ow[-1:]
                out.append(inst)
            blk.instructions = out
    return n_split


def _get_nc():
    if "nc" not in _NC_CACHE:
        _NC_CACHE["nc"] = _build_bass()
    return _NC_CACHE["nc"]


def _prep_core_inputs(x_core, weight_arrs, n_sc=N_SC, ch_b=CH_B):
    """x_core: [b_core, T, I] fp32 -> the per-core input map."""
    # [sc, chunk j, b, t, i]
    xr = x_core.reshape(n_sc, 4, ch_b, T_STEPS, I_DIM)
    # -> [sc, t, j, i, b]
    xf = np.ascontiguousarray(xr.transpose(0, 3, 1, 4, 2))
    xt = np.zeros((n_sc, N_TG, 4, KX, 4, ch_b), _BF16)
    for t in range(T_STEPS):
        tg, u = divmod(t, 4)
        xt[:, tg, :, 0:I_DIM, u, :] = xf[:, t].astype(_BF16)
        xt[:, tg, :, I_DIM, u, :] = _BF16(1.0)
    m = {"xt": xt.reshape(n_sc, N_TG, KB, 4, ch_b)}
    m.update(weight_arrs)
    return m


def _prep_weights(W_ih, W_hh, b_ih, b_hh, W_fc, b_fc):
    W_ih = np.asarray(W_ih, dtype=np.float32)
    W_hh = np.asarray(W_hh, dtype=np.float32)
    b = np.asarray(b_ih, dtype=np.float32) + np.asarray(b_hh, dtype=np.float32)
    W_fc = np.asarray(W_fc, dtype=np.float32)
    b_fc = np.asarray(b_fc, dtype=np.float32)

    wx = np.zeros((KB, 512), np.float32)
    wh = np.zeros((128, 512), np.float32)
    wfc = np.zeros((128, 128), np.float32)
    for g in range(4):
        pg = GATE_PERM[g]
        rows = slice(32 * pg, 32 * pg + 32)
        gs = 2.0 if g == 3 else 1.0  # tanh-as-sigmoid: double the g bank
        for j in range(4):
            wx[KX * j:KX * j + I_DIM, 128 * g + 32 * j:128 * g + 32 * j + 32] \
                = gs * W_ih[rows, :].T
            wx[KX * j + I_DIM, 128 * g + 32 * j:128 * g + 32 * j + 32] \
                = gs * b[rows]
            wh[32 * j:32 * j + 32, 128 * g + 32 * j:128 * g + 32 * j + 32] \
                = gs * W_hh[rows, :].T
    for j in range(4):
        wfc[32 * j:32 * j + 32, 32 * j:32 * j + C_DIM] = W_fc.T
    bfc = np.zeros((128, 1), np.float32)
    for j in range(4):
        bfc[32 * j:32 * j + C_DIM, 0] = b_fc
    return {
        "wx": wx.astype(_BF16),
        "wh": wh.astype(_BF16),
        "wfc": wfc.astype(_BF16),
        "bfc": bfc,
    }


def _run(inputs, trace=False):
    from concourse.bass_utils import run_bass_kernel_spmd

    nc = _get_nc()
    x = np.asarray(inputs["x"], dtype=np.float32)
    w = _prep_weights(inputs["W_ih"], inputs["W_hh"], inputs["b_ih"],
                      inputs["b_hh"], inputs["W_fc"], inputs["b_fc"])
    b_core = B_FULL // N_CORES
    in_maps = [
        _prep_core_inputs(x[i * b_core:(i + 1) * b_core], w)
        for i in range(N_CORES)
    ]
    last_err = None
    for attempt in range(4):
        try:
            res = run_bass_kernel_spmd(
                nc, in_maps, core_ids=list(range(N_CORES)), trace=trace,
            )
            break
        except Exception as e:  # transient device wedges: retry
            last_err = e
            import time as _time
            _time.sleep(3.0)
    else:
        raise last_err
    out = np.concatenate(
        [np.asarray(res.results[i]["out"]) for i in range(N_CORES)], axis=0
    )
    return out, res


def kernel(x, W_ih, W_hh, b_ih, b_hh, W_fc, b_fc):
    out, _ = _run(dict(x=x, W_ih=W_ih, W_hh=W_hh, b_ih=b_ih, b_hh=b_hh,
                       W_fc=W_fc, b_fc=b_fc))
    return out
